# revision 38
# baseline (speedup 1.0000x reference)
# Tensor-parallel GQA attention kernel for 8 Trainium2 NeuronCores.
#
# Sharding: each core owns 4 query heads + 1 kv head (32 q / 8 kv heads
# total), computes q/k/v projections for its heads, RoPE, causal
# attention, and a partial o_proj (row slice of wo); the host sums the 8
# partial outputs.
#
# Per-core layout: everything is kept "transposed" ([dim, seq]) so the
# contraction dim of every matmul is the partition axis:
#   qT = wq_c.T @ x.T        [256, S]   (scale folded into wq_c)
#   kvT = wkv_c.T @ x.T      [128, S]   (k rows 0:64, v rows 64:128)
#   scoresT[j, i] = kT.T q   [128-block j, 512-chunk i]  (K=64)
#   causal mask: an extra matmul accumulates -80 * (1 - mask) into the
#   scores psum ((-80 I).T @ cminv), so exp() of masked entries ~ 1e-33
#   attn_T[d, i] = [v|1].T @ exp(scores)   (row 64 = softmax denoms)
#   out_partial[s, :] = attn_T.T-contracted with wo_c rows (fp16 out,
#   host accumulates)
#
# v2 structure: attention runs in two head-WAVES per 512-query chunk
# (wave w covers heads 2w, 2w+1).  A wave's two heads share one
# [128, 1024] 2-bank score-psum tile, so ONE wide (strided) exp per
# (J-block, wave) halves the Scalar-engine instruction count - the
# scalar exp stream is the attention-phase bottleneck.  PSUM is split
# into pool `pq` (4 x 2KB rotating: psq / po accumulators / o_proj /
# selector) and pool `sc` (2 x 4KB: kv-projection pair / score pairs).
# PV matmuls lag scores by 4 J-blocks; the softmax finalize is split in
# two (fin1: DVE reciprocal chain; fin2: selector matmul + normalize)
# emitted a few J-blocks apart so the PE never waits on the reciprocal.
# o_proj units of chunk ic are spread through chunk ic+1's J-loops as
# dense PE filler; the last chunk's units evacuate via the (then-idle)
# Scalar engine.

import sys
from contextlib import ExitStack

for _p in ("/opt/trn_rl_repo", "/root/.axon_site"):
    if _p not in sys.path:
        sys.path.insert(0, _p)

import numpy as np

import concourse.bacc as bacc
import concourse.mybir as mybir
import concourse.tile as tile
from concourse.bass_utils import run_bass_kernel_spmd

F32 = mybir.dt.float32
EXP = mybir.ActivationFunctionType.Exp

# matmul operand dtype: float32r (2 cyc/row, ~1.5e-4/matmul) or
# float16 (1 cyc/row, ~7e-4/matmul)
MM_DT = mybir.dt.float16
MASK_NEG = 80.0

S = 2048          # sequence length
H = 2048          # hidden size
NH = 32           # query heads
NKV = 8           # kv heads
HD = 64           # head dim
NCORES = 8
HPC = NH // NCORES        # query heads per core = 4
DQ = HPC * HD             # per-core q width = 256
SCALE = HD ** -0.5
P = 128
NB = S // P               # 16 128-blocks along seq
NC4 = S // 512            # 4 512-chunks along seq
KCH = H // P              # 16 contraction chunks
HS = S // 2               # half seq


def _build_module(mm_dt):
    nc = bacc.Bacc(trn_type="TRN2", debug=False)

    xT_d = nc.dram_tensor("xT", [H, S], mm_dt, kind="ExternalInput").ap()
    wq_d = nc.dram_tensor("wq", [P, KCH * DQ], mm_dt, kind="ExternalInput").ap()
    wkv_d = nc.dram_tensor("wkv", [P, KCH * P], mm_dt, kind="ExternalInput").ap()
    wo_d = nc.dram_tensor("wo", [P, 2 * S], mm_dt, kind="ExternalInput").ap()
    cos_d = nc.dram_tensor("cos2", [P, S], mm_dt, kind="ExternalInput").ap()
    sin_d = nc.dram_tensor("sin2", [P, S], mm_dt, kind="ExternalInput").ap()
    # inverted causal masks for the 4 diagonal offsets, and -80*I, in
    # fp8e4 DoubleRow layout (two stacked 64-row k-tiles; all values are
    # exactly representable, and DoubleRow halves the mask matmul cost)
    F8 = mybir.dt.float8e4
    cm_d = nc.dram_tensor("cm8", [64, 2 * 4 * 512], F8, kind="ExternalInput").ap()
    ni_d = nc.dram_tensor("ni8", [64, 2 * P], F8, kind="ExternalInput").ap()
    # selector matrix for the denominator broadcast (per wave)
    ew_d = nc.dram_tensor("ew", [P, P], mm_dt, kind="ExternalInput").ap()
    out_d = nc.dram_tensor("out", [S, H], mm_dt, kind="ExternalOutput").ap()

    with tile.TileContext(nc) as tc, ExitStack() as ctx:
        pers = ctx.enter_context(tc.tile_pool(name="pers", bufs=1))

        wq_sb = pers.tile([P, KCH * DQ], mm_dt, tag="wq_sb", name="wq_sb")
        wkv_sb = pers.tile([P, KCH * P], mm_dt, tag="wkv_sb", name="wkv_sb")
        cos_sb = pers.tile([P, S], mm_dt, tag="cos_sb", name="cos_sb")
        sin_sb = pers.tile([P, S], mm_dt, tag="sin_sb", name="sin_sb")
        wo_sb = pers.tile([P, 2 * S], mm_dt, tag="wo_sb", name="wo_sb")
        cm_sb = pers.tile([64, 2 * 4 * 512], F8, tag="cm_sb", name="cm_sb")
        ni_sb = pers.tile([64, 2 * P], F8, tag="ni_sb", name="ni_sb")
        ew_sb = pers.tile([P, P], mm_dt, tag="ew_sb", name="ew_sb")

        ones16 = pers.tile([P, NB], mm_dt, tag="ones16", name="ones16")
        nc.vector.memset(ones16[:], 1.0)

        # per-half persistent tensors (w == m: wave w covers heads 2w,2w+1)
        qrh = [[pers.tile([P, HS], mm_dt, tag=f"qr{m}_{g}", name=f"qr{m}_{g}")
                for g in range(2)] for m in range(2)]
        ktdh = [pers.tile([P, HS], mm_dt, tag=f"ktd{g}", name=f"ktd{g}")
                for g in range(2)]
        vtmph = [pers.tile([64, HS], mm_dt, tag=f"vtmp{g}", name=f"vtmp{g}")
                 for g in range(2)]
        vbh = [pers.tile([P, 8 * (HD + 1)], mm_dt, tag=f"vb{g}", name=f"vb{g}")
               for g in range(2)]
        attn0 = pers.tile([P, S], mm_dt, tag="attn0", name="attn0")
        attn1 = pers.tile([P, S], mm_dt, tag="attn1", name="attn1")
        attn = [attn0, attn1]
        rsum = pers.tile([P, 512], F32, tag="rsum", name="rsum")
        nc.vector.memset(rsum[:], 1.0)
        rrf = pers.tile([P, 512], F32, tag="rrf", name="rrf")
        rrs = pers.tile([P, 512], F32, tag="rrs", name="rrs")
        rr = pers.tile([P, 512], mm_dt, tag="rr", name="rr")

        for g in range(2):
            vbv = vbh[g][:].rearrange("p (b c) -> p b c", c=HD + 1)
            nc.vector.tensor_copy(vbv[:, :, HD:HD + 1], ones16[:, 0:8])

        # pools; PSUM: pq = 4 x 2KB banks, sc = 2 x 4KB (2-bank) slots
        xp = ctx.enter_context(tc.tile_pool(name="xp", bufs=17))
        rs = ctx.enter_context(tc.tile_pool(name="rs", bufs=2))
        pq = ctx.enter_context(tc.tile_pool(name="pq", bufs=4, space="PSUM"))
        scp = ctx.enter_context(tc.tile_pool(name="scp", bufs=2, space="PSUM"))
        pxp = ctx.enter_context(tc.tile_pool(name="pxp", bufs=9))
        otp = ctx.enter_context(tc.tile_pool(name="otp", bufs=3))

        psq_store = {}
        pskv_store = {}
        xts_store = {}

        # ---------------- projections ----------------
        def proj_mm(g):
            # two passes over the k-chunks: q-matmuls and kv-matmuls are
            # split so the psum handoff to RoPE/the next phase overlaps
            # with the other projection's matmuls (g=0: q first, so the
            # psq banks release during the kv pass; g=1: kv first, so the
            # g=0 rope casts finish during the kv pass)
            nw = slice(1024 * g, 1024 * g + 1024)
            psq = [[pq.tile([P, 512], F32, tag="pq", name=f"psq{m}_{half}")
                    for half in range(2)] for m in range(2)]
            pskv = scp.tile([P, 1024], F32, tag="sc", name="pskv")
            psq_store[g] = psq
            pskv_store[g] = pskv

            def emit_q(k, xtile):
                st = dict(start=(k == 0), stop=(k == KCH - 1))
                for half in range(2):
                    o = 1024 * g + 512 * half
                    xs = xtile[:, o:o + 512]
                    nc.tensor.matmul(
                        psq[0][half][:], wq_sb[:, k * DQ:k * DQ + P], xs, **st)
                    nc.tensor.matmul(
                        psq[1][half][:], wq_sb[:, k * DQ + P:k * DQ + DQ],
                        xs, **st)

            def emit_kv(k, xtile):
                st = dict(start=(k == 0), stop=(k == KCH - 1))
                for half in range(2):
                    o = 1024 * g + 512 * half
                    xs = xtile[:, o:o + 512]
                    nc.tensor.matmul(
                        pskv[:, 512 * half:512 * half + 512],
                        wkv_sb[:, k * P:k * P + P], xs, **st)

            for k in range(KCH):
                if g == 0:
                    if k in (0, 1) or (k % 4 == 2 and k < 12):
                        # k=0/1 come alone so the first matmuls start early
                        if k == 0:
                            wqs, wks = slice(0, 256), slice(0, 128)
                        elif k == 1:
                            wqs, wks = slice(256, 1024), slice(128, 512)
                        else:
                            kg = k // 4 + 1
                            wqs = slice(1024 * kg, 1024 * kg + 1024)
                            wks = slice(512 * kg, 512 * kg + 512)
                        nc.sync.dma_start(wq_sb[:, wqs], wq_d[:, wqs])
                        nc.sync.dma_start(wkv_sb[:, wks], wkv_d[:, wks])
                    # full-row x tiles: both halves of x load once here;
                    # the g=1 projection then needs no x DMA at all
                    xt = xp.tile([P, S], mm_dt, tag="xt", name="xt")
                    xts_store[k] = xt
                    if k < 2:
                        # split the first tiles so data lands sooner
                        nc.sync.dma_start(
                            xt[:, 0:512], xT_d[P * k:P * k + P, 0:512])
                        nc.sync.dma_start(
                            xt[:, 512:1024], xT_d[P * k:P * k + P, 512:1024])
                        nc.sync.dma_start(
                            xt[:, 1024:2048],
                            xT_d[P * k:P * k + P, 1024:2048])
                    else:
                        nc.sync.dma_start(xt[:], xT_d[P * k:P * k + P, :])
                if g == 0:
                    emit_q(k, xts_store[k])
                else:
                    emit_kv(k, xts_store[k])
                if g == 0 and k == 3:
                    # trig tables needed by the first RoPE below
                    nc.sync.dma_start(cos_sb[:], cos_d[:])
                    nc.sync.dma_start(sin_sb[:], sin_d[:])
                if g == 0 and k == 9:
                    nc.sync.dma_start(cm_sb[:], cm_d[:])
                    nc.sync.dma_start(ni_sb[:], ni_d[:])
                    nc.sync.dma_start(ew_sb[:], ew_d[:])
                if g == 1 and k == 4:
                    # wo is first needed by chunk-1's o_proj fillers; keep
                    # its 2MB out of the projection-critical DMA window
                    nc.sync.dma_start(wo_sb[:], wo_d[:])
            for k in range(KCH):
                if g == 0:
                    emit_kv(k, xts_store[k])
                else:
                    emit_q(k, xts_store[k])
                    del xts_store[k]

        # ---------------- RoPE ----------------
        def rope(g):
            # PSUM evacuation is split across Scalar and DVE (fp16
            # casts); for g=0 the q casts go first (the psq slots gate
            # the g=1 projection), for g=1 the kv casts go on DVE (they
            # gate the chunk-0 score tiles and must not queue behind the
            # chunk-0 exps on the scalar engine).  Then an SBUF-only
            # fp16 rotate-half chain on DVE.
            nw = slice(1024 * g, 1024 * g + 1024)
            psq = psq_store.pop(g)
            pskv = pskv_store.pop(g)
            q16s = []
            for m in range(2):
                q16 = rs.tile([P, 1024], mm_dt, tag=f"q16_{m}", name=f"q16_{m}")
                q16s.append(q16)
                eng = nc.vector if (g == 0 and m == 0) else nc.scalar
                for half in range(2):
                    hs = slice(512 * half, 512 * half + 512)
                    if eng is nc.vector:
                        nc.vector.tensor_copy(q16[:, hs], psq[m][half][:])
                    else:
                        nc.scalar.copy(q16[:, hs], psq[m][half][:])
            k16 = rs.tile([64, 1024], mm_dt, tag="k16", name="k16")
            for half in range(2):
                hs = slice(512 * half, 512 * half + 512)
                if g == 1:
                    nc.vector.tensor_copy(k16[:, hs], pskv[0:64, hs])
                else:
                    nc.scalar.copy(k16[:, hs], pskv[0:64, hs])
            for half in range(2):
                hs = slice(512 * half, 512 * half + 512)
                if g == 1:
                    nc.vector.tensor_copy(vtmph[g][:, hs], pskv[64:P, hs])
                else:
                    nc.scalar.copy(vtmph[g][:, hs], pskv[64:P, hs])
            # fp16 SBUF chains on DVE
            for m in range(2):
                q16 = q16s[m]
                nc.vector.tensor_mul(qrh[m][g][:], q16[:], cos_sb[:, nw])
                qsw = rs.tile([P, 1024], mm_dt, tag=f"qsw{m}", name=f"qsw{m}")
                for b0 in (0, 64):
                    nc.vector.tensor_copy(
                        qsw[b0:b0 + 32, :], q16[b0 + 32:b0 + 64, :])
                    nc.vector.tensor_copy(
                        qsw[b0 + 32:b0 + 64, :], q16[b0:b0 + 32, :])
                nc.vector.tensor_mul(qsw[:], qsw[:], sin_sb[:, nw])
                nc.vector.tensor_add(qrh[m][g][:], qrh[m][g][:], qsw[:])
            nc.vector.tensor_mul(ktdh[g][0:64, :], k16[:], cos_sb[0:64, nw])
            ksw = rs.tile([64, 1024], mm_dt, tag="ksw", name="ksw")
            nc.vector.tensor_copy(ksw[0:32, :], k16[32:64, :])
            nc.vector.tensor_copy(ksw[32:64, :], k16[0:32, :])
            nc.vector.tensor_mul(ksw[:], ksw[:], sin_sb[0:64, nw])
            nc.vector.tensor_add(ktdh[g][0:64, :], ktdh[g][0:64, :], ksw[:])
            # duplicate k rows for the upper-head score matmuls
            nc.vector.tensor_copy(ktdh[g][64:P, :], ktdh[g][0:64, :])
            # stream-transpose v into vb blocks (DVE)
            vbv = vbh[g][:].rearrange("p (b c) -> p b c", c=HD + 1)
            vtv = vtmph[g][:].rearrange("p (b c) -> p b c", c=P)
            for a in range(4):
                for b in range(2):
                    nc.vector.transpose(
                        vbv[32 * a:32 * a + 32, :, 32 * b:32 * b + 32],
                        vtv[32 * b:32 * b + 32, :, 32 * a:32 * a + 32])

        # ---------------- attention ----------------
        pxstore = {}

        def emit_scores(ic, J, w):
            icg, icr = ic // 2, ic % 2
            t = J - 4 * ic
            c0 = 128 * t if t > 0 else 0
            Jg, Jr = J // 8, J % 8
            Js = slice(P * Jr, P * Jr + P)
            qs = slice(512 * icr + c0, 512 * icr + 512)
            ps_s = scp.tile([P, 1024], F32, tag="sc", name="ps_s")
            for hh in range(2):
                b0, col = 64 * hh, 512 * hh
                nc.tensor.matmul(
                    ps_s[:, col + c0:col + 512], ktdh[Jg][b0:b0 + 64, Js],
                    qrh[w][icg][b0:b0 + 64, qs],
                    start=True, stop=(t < 0))
            if t >= 0:
                ni_v = ni_sb[:].rearrange("p (two m) -> p two m", two=2)
                cm_v = cm_sb[:].rearrange("p (two tc) -> p two tc", two=2)
                for hh in range(2):
                    col = 512 * hh
                    nc.tensor.matmul(
                        ps_s[:, col + c0:col + 512], ni_v,
                        cm_v[:, :, 512 * t + c0:512 * t + 512],
                        start=False, stop=True,
                        perf_mode=mybir.MatmulPerfMode.DoubleRow)
            px = pxp.tile([P, 1024], mm_dt, tag="pxp", name="px")
            # one strided exp covering both heads' live regions
            ps_v = ps_s[:].rearrange("p (two c) -> p two c", two=2)
            px_v = px[:].rearrange("p (two c) -> p two c", two=2)
            nc.scalar.activation(px_v[:, :, c0:], ps_v[:, :, c0:], EXP)
            pxstore[(ic, J, w)] = px

        def emit_pv(ic, J, w, po_w):
            t = J - 4 * ic
            c0 = 128 * t if t > 0 else 0
            nJ = 4 * ic + 4
            Jg, Jr = J // 8, J % 8
            vs = slice((HD + 1) * Jr, (HD + 1) * Jr + HD + 1)
            px = pxstore.pop((ic, J, w))
            for hh in range(2):
                col = 512 * hh
                nc.tensor.matmul(
                    po_w[hh][:, c0:], vbh[Jg][:, vs], px[:, col + c0:col + 512],
                    start=(J == 0), stop=(J == nJ - 1))

        def fin1(ic, w, po_w):
            # DVE-only: gather the 2 denominator rows, batched reciprocal
            # (fast variant: ~18 correct bits, far above the fp16 rr cast)
            for hh in range(2):
                nc.vector.tensor_copy(
                    rsum[32 * hh:32 * hh + 1, :], po_w[hh][HD:HD + 1, :])
            with nc.allow_low_precision(reason="softmax reciprocal"):
                nc.vector.reciprocal_approx_fast(rrf[:], rsum[:])
                nc.vector.tensor_copy(rr[:], rrf[:])

        def fin2(ic, w, po_w):
            # selector matmul broadcasts the reciprocals, then normalize
            psb = pq.tile([P, 512], F32, tag="pq", name="psb")
            nc.tensor.matmul(psb[:], ew_sb[:], rr[:], start=True, stop=True)
            psbs = rs.tile([P, 512], F32, tag="psbs", name="psbs")
            nc.vector.tensor_copy(psbs[:], psb[:])
            for hh in range(2):
                b0 = 64 * hh
                asl = attn[w][b0:b0 + 64, 512 * ic:512 * ic + 512]
                nc.vector.tensor_mul(asl, po_w[hh][0:HD, :], psbs[b0:b0 + 64, :])

        ot_store = {}

        def emit_oproj_unit(sb, n4, tail=False):
            ss = slice(P * sb, P * sb + P)
            ps_o = pq.tile([P, 512], F32, tag="pq", name="ps_o")
            nc.tensor.matmul(
                ps_o[:], attn0[:, ss],
                wo_sb[:, 512 * n4:512 * n4 + 512],
                start=True, stop=False)
            nc.tensor.matmul(
                ps_o[:], attn1[:, ss],
                wo_sb[:, S + 512 * n4:S + 512 * n4 + 512],
                start=False, stop=True)
            if n4 == 0:
                ot_store[sb] = otp.tile([P, H], mm_dt, tag="otp", name="ot")
            ot = ot_store[sb]
            osl = slice(512 * n4, 512 * n4 + 512)
            if tail:
                # both scalar and DVE are idle at the tail: alternate casts
                if n4 % 2 == 0:
                    nc.scalar.copy(ot[:, osl], ps_o[:])
                else:
                    nc.vector.tensor_copy(ot[:, osl], ps_o[:])
            else:
                nc.vector.tensor_copy(ot[:, osl], ps_o[:])
            if n4 == NC4 - 1:
                nc.sync.dma_start(out_d[ss, :], ot_store.pop(sb)[:])

        # ---------------- emission ----------------
        proj_mm(0)
        rope(0)
        proj_mm(1)
        # chunk-0's first score groups (and their exps) are emitted before
        # rope(1) so the scalar engine starts the exp stream immediately
        # after the g=1 projection instead of behind rope(1)'s casts
        emit_scores(0, 0, 0)
        emit_scores(0, 1, 0)
        rope(1)
        pending_fin2 = None
        units = []
        ui = si = 0
        slots_total = 1

        for ic in range(NC4):
            nJ = 4 * ic + 4
            # filler units: o_proj of the previous chunk, spread over this
            # chunk's score/PV loop iterations (skipping the first two of
            # each wave, which cover fin2 / chunk-boundary latency)
            units = [(sb, n4) for sb in range(4 * (ic - 1), 4 * ic)
                     for n4 in range(NC4)] if ic > 0 else []
            ui = si = 0
            slots_total = max(1, 2 * (nJ - 5))
            for w in range(2):
                po_w = None
                npv = 0
                iters = list(range(2, nJ))
                # lookahead target: first two score groups of the next
                # wave/chunk, emitted mid-loop (long waves) so their exps
                # are already drained when the next wave's PVs need them
                if w == 0:
                    nxt = (ic, 1)
                elif ic + 1 < NC4:
                    nxt = (ic + 1, 0)
                else:
                    nxt = None
                inloop_la = len(iters) >= 6
                for idx, J in enumerate(iters):
                    emit_scores(ic, J, w)
                    if idx == 0 and pending_fin2 is not None:
                        fin2(*pending_fin2)
                        pending_fin2 = None
                    if idx >= 2:
                        if po_w is None:
                            po_w = [pq.tile([HD + 1, 512], F32, tag="pq",
                                            name=f"po{ic}_{w}_{hh}")
                                    for hh in range(2)]
                        emit_pv(ic, npv, w, po_w)
                        npv += 1
                        if inloop_la and nxt is not None and \
                                idx in (len(iters) - 4, len(iters) - 3):
                            emit_scores(nxt[0], idx - (len(iters) - 4), nxt[1])
                    # fillers skip the wave's last iteration so the
                    # DVE is free for the softmax-finalize chain
                    if 2 <= idx < len(iters) - 1:
                        si += 1
                        take = (len(units) * si) // slots_total - ui
                        while take > 0 and ui < len(units):
                            emit_oproj_unit(*units[ui])
                            ui += 1
                            take -= 1
                if po_w is None:
                    po_w = [pq.tile([HD + 1, 512], F32, tag="pq",
                                    name=f"po{ic}_{w}_{hh}")
                            for hh in range(2)]
                while npv < nJ:
                    emit_pv(ic, npv, w, po_w)
                    npv += 1
                fin1(ic, w, po_w)
                if nxt is not None and not inloop_la:
                    emit_scores(nxt[0], 0, nxt[1])
                    emit_scores(nxt[0], 1, nxt[1])
                pending_fin2 = (ic, w, po_w)
            while ui < len(units):
                emit_oproj_unit(*units[ui])
                ui += 1
        # tail: finalize the last wave, then its o_proj via scalar casts
        fin2(*pending_fin2)
        for sb in range(12, 16):
            for n4 in range(NC4):
                emit_oproj_unit(sb, n4, tail=True)

    nc.compile()
    return nc


_NC_CACHE = {}


def _get_module(mm_dt=MM_DT):
    if mm_dt not in _NC_CACHE:
        _NC_CACHE[mm_dt] = _build_module(mm_dt)
    return _NC_CACHE[mm_dt]


def _prep_inputs(x, wq, wk, wv, wo, cos, sin, mm_dt=MM_DT):
    mm_np = mybir.dt.np(mm_dt)
    x = np.asarray(x, dtype=np.float32)
    xT = np.ascontiguousarray(x.reshape(S, H).T.astype(mm_np))

    cosT = np.asarray(cos, dtype=np.float32).T          # [64, S]
    sinT = np.asarray(sin, dtype=np.float32).T          # [64, S]
    sgn = np.where(np.arange(HD) < HD // 2, -1.0, 1.0).astype(np.float32)
    sinT_s = sinT * sgn[:, None]
    cos2 = np.ascontiguousarray(np.tile(cosT, (2, 1))).astype(mm_np)  # [128, S]
    sin2 = np.ascontiguousarray(np.tile(sinT_s, (2, 1))).astype(mm_np)

    # inverted causal masks (1 where masked out), diagonal offsets 0..3,
    # in fp8e4 DoubleRow layout: [64 rows, (ktile two, 4 offsets, 512)]
    np8 = mybir.dt.np(mybir.dt.float8e4)
    jl = np.arange(P)[:, None]
    il = np.arange(512)[None, :]
    cm8 = np.zeros((64, 2, 4, 512), dtype=np.float32)
    for t in range(4):
        blk = (jl + P * t > il).astype(np.float32)
        cm8[:, 0, t, :] = blk[0:64]
        cm8[:, 1, t, :] = blk[64:128]
    cm8 = np.ascontiguousarray(cm8.reshape(64, 4096)).astype(np8)
    ni8 = np.zeros((64, 2, P), dtype=np.float32)
    for j in range(64):
        ni8[j, 0, j] = -MASK_NEG
        ni8[j, 1, 64 + j] = -MASK_NEG
    ni8 = np.ascontiguousarray(ni8.reshape(64, 2 * P)).astype(np8)

    # selector matrix: psb rows 0:64 get the reciprocal row of the wave's
    # first head (partition 0), rows 64:128 the second head (partition 32)
    ew = np.zeros((P, P), dtype=np.float32)
    ew[0, 0:64] = 1.0
    ew[32, 64:128] = 1.0
    ew = ew.astype(mm_np)

    def chunk_kxm(w):
        # [H, M] -> [128, KCH*M] with k-chunk-major free layout
        m = w.shape[1]
        return np.ascontiguousarray(
            w.reshape(KCH, P, m).transpose(1, 0, 2).reshape(P, KCH * m).astype(mm_np))

    wq = np.asarray(wq, dtype=np.float32)
    wk = np.asarray(wk, dtype=np.float32)
    wv = np.asarray(wv, dtype=np.float32)
    wo = np.asarray(wo, dtype=np.float32)

    in_maps = []
    for c in range(NCORES):
        wq_c = wq[:, DQ * c:DQ * c + DQ] * SCALE
        wkv_c = np.concatenate(
            [wk[:, HD * c:HD * c + HD], wv[:, HD * c:HD * c + HD]], axis=1)
        wo_c = wo[DQ * c:DQ * c + DQ, :]
        wo_l = np.ascontiguousarray(
            wo_c.reshape(2, P, H).transpose(1, 0, 2).reshape(P, 2 * H).astype(mm_np))
        in_maps.append({
            "xT": xT,
            "wq": chunk_kxm(wq_c),
            "wkv": chunk_kxm(wkv_c),
            "wo": wo_l,
            "cos2": cos2,
            "sin2": sin2,
            "cm8": cm8,
            "ni8": ni8,
            "ew": ew,
        })
    return in_maps


def run(inputs, trace=False, trace_kwargs=None, mm_dt=MM_DT):
    """Execute on 8 cores; returns (full_output, BassKernelResults)."""
    nc = _get_module(mm_dt)
    in_maps = _prep_inputs(
        inputs["x"], inputs["wq"], inputs["wk"], inputs["wv"],
        inputs["wo"], inputs["cos"], inputs["sin"], mm_dt=mm_dt)
    kwargs = {}
    if trace:
        kwargs = dict(trace=True, **(trace_kwargs or {}))
    res = run_bass_kernel_spmd(nc, in_maps, core_ids=list(range(NCORES)), **kwargs)
    acc = np.zeros((S, H), dtype=np.float32)
    for c in range(NCORES):
        acc += res.results[c]["out"].astype(np.float32)
    out = acc.reshape(1, S, H)
    return out, res


def kernel(**inputs):
    out, _ = run(inputs, trace=False)
    return out


# revision 41
# speedup vs baseline: 1.0543x; 1.0543x over previous
# Tensor-parallel GQA attention kernel for 8 Trainium2 NeuronCores.
#
# Sharding: each core owns 4 query heads + 1 kv head (32 q / 8 kv heads
# total), computes q/k/v projections for its heads, RoPE, causal
# attention, and a partial o_proj (row slice of wo); the host sums the 8
# partial outputs.
#
# Per-core layout: everything is kept "transposed" ([dim, seq]) so the
# contraction dim of every matmul is the partition axis:
#   qT = wq_c.T @ x.T        [256, S]   (scale folded into wq_c)
#   kvT = wkv_c.T @ x.T      [128, S]   (k rows 0:64, v rows 64:128)
#   scoresT[j, i] = kT.T q   [128-block j, 512-chunk i]  (K=64)
#   causal mask: an extra matmul accumulates -80 * (1 - mask) into the
#   scores psum ((-80 I).T @ cminv), so exp() of masked entries ~ 1e-33
#   attn_T[d, i] = [v|1].T @ exp(scores)   (row 64 = softmax denoms)
#   out_partial[s, :] = attn_T.T-contracted with wo_c rows (fp16 out,
#   host accumulates)
#
# v2 structure: attention runs in two head-WAVES per 512-query chunk
# (wave w covers heads 2w, 2w+1).  A wave's two heads share one
# [128, 1024] 2-bank score-psum tile, so ONE wide (strided) exp per
# (J-block, wave) halves the Scalar-engine instruction count - the
# scalar exp stream is the attention-phase bottleneck.  PSUM is split
# into pool `pq` (4 x 2KB rotating: psq / po accumulators / o_proj /
# selector) and pool `sc` (2 x 4KB: kv-projection pair / score pairs).
# PV matmuls lag scores by 4 J-blocks; the softmax finalize is split in
# two (fin1: DVE reciprocal chain; fin2: selector matmul + normalize)
# emitted a few J-blocks apart so the PE never waits on the reciprocal.
# o_proj units of chunk ic are spread through chunk ic+1's J-loops as
# dense PE filler; the last chunk's units evacuate via the (then-idle)
# Scalar engine.

import sys
from contextlib import ExitStack

for _p in ("/opt/trn_rl_repo", "/root/.axon_site"):
    if _p not in sys.path:
        sys.path.insert(0, _p)

import numpy as np

import concourse.bacc as bacc
import concourse.mybir as mybir
import concourse.tile as tile
from concourse.bass_utils import run_bass_kernel_spmd

F32 = mybir.dt.float32
EXP = mybir.ActivationFunctionType.Exp

# matmul operand dtype: float32r (2 cyc/row, ~1.5e-4/matmul) or
# float16 (1 cyc/row, ~7e-4/matmul)
MM_DT = mybir.dt.float16
MASK_NEG = 80.0

S = 2048          # sequence length
H = 2048          # hidden size
NH = 32           # query heads
NKV = 8           # kv heads
HD = 64           # head dim
NCORES = 8
HPC = NH // NCORES        # query heads per core = 4
DQ = HPC * HD             # per-core q width = 256
SCALE = HD ** -0.5
P = 128
NB = S // P               # 16 128-blocks along seq
NC4 = S // 512            # 4 512-chunks along seq
KCH = H // P              # 16 contraction chunks
HS = S // 2               # half seq


def _build_module(mm_dt):
    nc = bacc.Bacc(trn_type="TRN2", debug=False)

    xT_d = nc.dram_tensor("xT", [H, S], mm_dt, kind="ExternalInput").ap()
    wq_d = nc.dram_tensor("wq", [P, KCH * DQ], mm_dt, kind="ExternalInput").ap()
    wkv_d = nc.dram_tensor("wkv", [P, KCH * P], mm_dt, kind="ExternalInput").ap()
    wo_d = nc.dram_tensor("wo", [P, 2 * S], mm_dt, kind="ExternalInput").ap()
    cos_d = nc.dram_tensor("cos2", [P, S], mm_dt, kind="ExternalInput").ap()
    sin_d = nc.dram_tensor("sin2", [P, S], mm_dt, kind="ExternalInput").ap()
    # inverted causal masks for the 4 diagonal offsets, and -80*I, in
    # fp8e4 DoubleRow layout (two stacked 64-row k-tiles; all values are
    # exactly representable, and DoubleRow halves the mask matmul cost)
    F8 = mybir.dt.float8e4
    cm_d = nc.dram_tensor("cm8", [64, 2 * 4 * 512], F8, kind="ExternalInput").ap()
    ni_d = nc.dram_tensor("ni8", [64, 2 * P], F8, kind="ExternalInput").ap()
    # selector matrix for the denominator broadcast (per wave)
    ew_d = nc.dram_tensor("ew", [P, P], mm_dt, kind="ExternalInput").ap()
    out_d = nc.dram_tensor("out", [S, H], mm_dt, kind="ExternalOutput").ap()

    with tile.TileContext(nc) as tc, ExitStack() as ctx:
        pers = ctx.enter_context(tc.tile_pool(name="pers", bufs=1))

        wq_sb = pers.tile([P, KCH * DQ], mm_dt, tag="wq_sb", name="wq_sb")
        wkv_sb = pers.tile([P, KCH * P], mm_dt, tag="wkv_sb", name="wkv_sb")
        cos_sb = pers.tile([P, S], mm_dt, tag="cos_sb", name="cos_sb")
        sin_sb = pers.tile([P, S], mm_dt, tag="sin_sb", name="sin_sb")
        wo_sb = pers.tile([P, 2 * S], mm_dt, tag="wo_sb", name="wo_sb")
        cm_sb = pers.tile([64, 2 * 4 * 512], F8, tag="cm_sb", name="cm_sb")
        ni_sb = pers.tile([64, 2 * P], F8, tag="ni_sb", name="ni_sb")
        ew_sb = pers.tile([P, P], mm_dt, tag="ew_sb", name="ew_sb")

        ones16 = pers.tile([P, NB], mm_dt, tag="ones16", name="ones16")
        nc.vector.memset(ones16[:], 1.0)

        # per-half persistent tensors (w == m: wave w covers heads 2w,2w+1)
        qrh = [[pers.tile([P, HS], mm_dt, tag=f"qr{m}_{g}", name=f"qr{m}_{g}")
                for g in range(2)] for m in range(2)]
        ktdh = [pers.tile([P, HS], mm_dt, tag=f"ktd{g}", name=f"ktd{g}")
                for g in range(2)]
        vtmph = [pers.tile([64, HS], mm_dt, tag=f"vtmp{g}", name=f"vtmp{g}")
                 for g in range(2)]
        vbh = [pers.tile([P, 8 * (HD + 1)], mm_dt, tag=f"vb{g}", name=f"vb{g}")
               for g in range(2)]
        attn0 = pers.tile([P, S], mm_dt, tag="attn0", name="attn0")
        attn1 = pers.tile([P, S], mm_dt, tag="attn1", name="attn1")
        attn = [attn0, attn1]
        rsum = pers.tile([P, 512], F32, tag="rsum", name="rsum")
        nc.vector.memset(rsum[:], 1.0)
        rrf = pers.tile([P, 512], F32, tag="rrf", name="rrf")
        rrs = pers.tile([P, 512], F32, tag="rrs", name="rrs")
        rr = pers.tile([P, 512], mm_dt, tag="rr", name="rr")

        for g in range(2):
            vbv = vbh[g][:].rearrange("p (b c) -> p b c", c=HD + 1)
            nc.vector.tensor_copy(vbv[:, :, HD:HD + 1], ones16[:, 0:8])

        # pools; PSUM: pq = 4 x 2KB banks, sc = 2 x 4KB (2-bank) slots
        xp = ctx.enter_context(tc.tile_pool(name="xp", bufs=17))
        rs = ctx.enter_context(tc.tile_pool(name="rs", bufs=2))
        pq = ctx.enter_context(tc.tile_pool(name="pq", bufs=4, space="PSUM"))
        scp = ctx.enter_context(tc.tile_pool(name="scp", bufs=2, space="PSUM"))
        pxp = ctx.enter_context(tc.tile_pool(name="pxp", bufs=9))
        otp = ctx.enter_context(tc.tile_pool(name="otp", bufs=3))

        psq_store = {}
        pskv_store = {}
        xts_store = {}

        # ---------------- projections ----------------
        def proj_mm(g):
            # two passes over the k-chunks: q-matmuls and kv-matmuls are
            # split so the psum handoff to RoPE/the next phase overlaps
            # with the other projection's matmuls (g=0: q first, so the
            # psq banks release during the kv pass; g=1: kv first, so the
            # g=0 rope casts finish during the kv pass)
            nw = slice(1024 * g, 1024 * g + 1024)
            psq = [[pq.tile([P, 512], F32, tag="pq", name=f"psq{m}_{half}")
                    for half in range(2)] for m in range(2)]
            pskv = scp.tile([P, 1024], F32, tag="sc", name="pskv")
            psq_store[g] = psq
            pskv_store[g] = pskv

            def emit_q(k, xtile):
                st = dict(start=(k == 0), stop=(k == KCH - 1))
                for half in range(2):
                    xs = xtile[:, 512 * half:512 * half + 512]
                    nc.tensor.matmul(
                        psq[0][half][:], wq_sb[:, k * DQ:k * DQ + P], xs, **st)
                    nc.tensor.matmul(
                        psq[1][half][:], wq_sb[:, k * DQ + P:k * DQ + DQ],
                        xs, **st)

            def emit_kv(k, xtile):
                st = dict(start=(k == 0), stop=(k == KCH - 1))
                for half in range(2):
                    xs = xtile[:, 512 * half:512 * half + 512]
                    nc.tensor.matmul(
                        pskv[:, 512 * half:512 * half + 512],
                        wkv_sb[:, k * P:k * P + P], xs, **st)

            for k in range(KCH):
                if g == 0 and (k in (0, 1) or (k % 4 == 2 and k < 12)):
                    # k=0/1 come alone so the first matmuls start early
                    if k == 0:
                        wqs, wks = slice(0, 256), slice(0, 128)
                    elif k == 1:
                        wqs, wks = slice(256, 1024), slice(128, 512)
                    else:
                        kg = k // 4 + 1
                        wqs = slice(1024 * kg, 1024 * kg + 1024)
                        wks = slice(512 * kg, 512 * kg + 512)
                    nc.sync.dma_start(wq_sb[:, wqs], wq_d[:, wqs])
                    nc.sync.dma_start(wkv_sb[:, wks], wkv_d[:, wks])
                # per-half x tiles, loaded in each g's first pass and
                # reused by its second pass
                xt = xp.tile([P, 1024], mm_dt, tag="xt", name="xt")
                xts_store[k] = xt
                if g == 0 and k < 2:
                    # split the first tiles so data lands sooner
                    nc.sync.dma_start(
                        xt[:, 0:512], xT_d[P * k:P * k + P, 0:512])
                    nc.sync.dma_start(
                        xt[:, 512:1024], xT_d[P * k:P * k + P, 512:1024])
                else:
                    nc.sync.dma_start(xt[:], xT_d[P * k:P * k + P, nw])
                if g == 0:
                    emit_q(k, xt)
                else:
                    emit_kv(k, xt)
                if g == 0 and k == 3:
                    # trig tables needed by the first RoPE below
                    nc.sync.dma_start(cos_sb[:], cos_d[:])
                    nc.sync.dma_start(sin_sb[:], sin_d[:])
                if g == 0 and k == 9:
                    nc.sync.dma_start(cm_sb[:], cm_d[:])
                    nc.sync.dma_start(ni_sb[:], ni_d[:])
                    nc.sync.dma_start(ew_sb[:], ew_d[:])
                if g == 1 and k == 4:
                    # wo is first needed by chunk-1's o_proj fillers; keep
                    # its 2MB out of the projection-critical DMA window
                    nc.sync.dma_start(wo_sb[:], wo_d[:])
            for k in range(KCH):
                if g == 0:
                    emit_kv(k, xts_store[k])
                else:
                    emit_q(k, xts_store[k])
                del xts_store[k]

        # ---------------- RoPE ----------------
        def rope(g):
            # PSUM evacuation is split across Scalar and DVE (fp16
            # casts); for g=0 the q casts go first (the psq slots gate
            # the g=1 projection), for g=1 the kv casts go on DVE (they
            # gate the chunk-0 score tiles and must not queue behind the
            # chunk-0 exps on the scalar engine).  Then an SBUF-only
            # fp16 rotate-half chain on DVE.
            nw = slice(1024 * g, 1024 * g + 1024)
            psq = psq_store.pop(g)
            pskv = pskv_store.pop(g)
            q16s = []
            for m in range(2):
                q16 = rs.tile([P, 1024], mm_dt, tag=f"q16_{m}", name=f"q16_{m}")
                q16s.append(q16)
                eng = nc.vector if (g == 0 and m == 0) else nc.scalar
                for half in range(2):
                    hs = slice(512 * half, 512 * half + 512)
                    if eng is nc.vector:
                        nc.vector.tensor_copy(q16[:, hs], psq[m][half][:])
                    else:
                        nc.scalar.copy(q16[:, hs], psq[m][half][:])
            k16 = rs.tile([64, 1024], mm_dt, tag="k16", name="k16")
            for half in range(2):
                hs = slice(512 * half, 512 * half + 512)
                if g == 1:
                    nc.vector.tensor_copy(k16[:, hs], pskv[0:64, hs])
                else:
                    nc.scalar.copy(k16[:, hs], pskv[0:64, hs])
            for half in range(2):
                hs = slice(512 * half, 512 * half + 512)
                if g == 1:
                    nc.vector.tensor_copy(vtmph[g][:, hs], pskv[64:P, hs])
                else:
                    nc.scalar.copy(vtmph[g][:, hs], pskv[64:P, hs])
            # fp16 SBUF chains on DVE
            for m in range(2):
                q16 = q16s[m]
                nc.vector.tensor_mul(qrh[m][g][:], q16[:], cos_sb[:, nw])
                qsw = rs.tile([P, 1024], mm_dt, tag=f"qsw{m}", name=f"qsw{m}")
                for b0 in (0, 64):
                    nc.vector.tensor_copy(
                        qsw[b0:b0 + 32, :], q16[b0 + 32:b0 + 64, :])
                    nc.vector.tensor_copy(
                        qsw[b0 + 32:b0 + 64, :], q16[b0:b0 + 32, :])
                nc.vector.tensor_mul(qsw[:], qsw[:], sin_sb[:, nw])
                nc.vector.tensor_add(qrh[m][g][:], qrh[m][g][:], qsw[:])
            nc.vector.tensor_mul(ktdh[g][0:64, :], k16[:], cos_sb[0:64, nw])
            ksw = rs.tile([64, 1024], mm_dt, tag="ksw", name="ksw")
            nc.vector.tensor_copy(ksw[0:32, :], k16[32:64, :])
            nc.vector.tensor_copy(ksw[32:64, :], k16[0:32, :])
            nc.vector.tensor_mul(ksw[:], ksw[:], sin_sb[0:64, nw])
            nc.vector.tensor_add(ktdh[g][0:64, :], ktdh[g][0:64, :], ksw[:])
            # duplicate k rows for the upper-head score matmuls
            nc.vector.tensor_copy(ktdh[g][64:P, :], ktdh[g][0:64, :])
            # stream-transpose v into vb blocks (DVE)
            vbv = vbh[g][:].rearrange("p (b c) -> p b c", c=HD + 1)
            vtv = vtmph[g][:].rearrange("p (b c) -> p b c", c=P)
            for a in range(4):
                for b in range(2):
                    nc.vector.transpose(
                        vbv[32 * a:32 * a + 32, :, 32 * b:32 * b + 32],
                        vtv[32 * b:32 * b + 32, :, 32 * a:32 * a + 32])

        # ---------------- attention ----------------
        pxstore = {}

        def emit_scores(ic, J, w):
            icg, icr = ic // 2, ic % 2
            t = J - 4 * ic
            c0 = 128 * t if t > 0 else 0
            Jg, Jr = J // 8, J % 8
            Js = slice(P * Jr, P * Jr + P)
            qs = slice(512 * icr + c0, 512 * icr + 512)
            ps_s = scp.tile([P, 1024], F32, tag="sc", name="ps_s")
            for hh in range(2):
                b0, col = 64 * hh, 512 * hh
                nc.tensor.matmul(
                    ps_s[:, col + c0:col + 512], ktdh[Jg][b0:b0 + 64, Js],
                    qrh[w][icg][b0:b0 + 64, qs],
                    start=True, stop=(t < 0))
            if t >= 0:
                ni_v = ni_sb[:].rearrange("p (two m) -> p two m", two=2)
                cm_v = cm_sb[:].rearrange("p (two tc) -> p two tc", two=2)
                for hh in range(2):
                    col = 512 * hh
                    nc.tensor.matmul(
                        ps_s[:, col + c0:col + 512], ni_v,
                        cm_v[:, :, 512 * t + c0:512 * t + 512],
                        start=False, stop=True,
                        perf_mode=mybir.MatmulPerfMode.DoubleRow)
            px = pxp.tile([P, 1024], mm_dt, tag="pxp", name="px")
            # one strided exp covering both heads' live regions
            ps_v = ps_s[:].rearrange("p (two c) -> p two c", two=2)
            px_v = px[:].rearrange("p (two c) -> p two c", two=2)
            nc.scalar.activation(px_v[:, :, c0:], ps_v[:, :, c0:], EXP)
            pxstore[(ic, J, w)] = px

        def emit_pv(ic, J, w, po_w):
            t = J - 4 * ic
            c0 = 128 * t if t > 0 else 0
            nJ = 4 * ic + 4
            Jg, Jr = J // 8, J % 8
            vs = slice((HD + 1) * Jr, (HD + 1) * Jr + HD + 1)
            px = pxstore.pop((ic, J, w))
            for hh in range(2):
                col = 512 * hh
                nc.tensor.matmul(
                    po_w[hh][:, c0:], vbh[Jg][:, vs], px[:, col + c0:col + 512],
                    start=(J == 0), stop=(J == nJ - 1))

        def fin1(ic, w, po_w):
            # DVE-only: gather the 2 denominator rows, batched reciprocal
            # (fast variant: ~18 correct bits, far above the fp16 rr cast)
            for hh in range(2):
                nc.vector.tensor_copy(
                    rsum[32 * hh:32 * hh + 1, :], po_w[hh][HD:HD + 1, :])
            with nc.allow_low_precision(reason="softmax reciprocal"):
                nc.vector.reciprocal_approx_fast(rrf[:], rsum[:])
                nc.vector.tensor_copy(rr[:], rrf[:])

        def fin2(ic, w, po_w):
            # selector matmul broadcasts the reciprocals, then normalize
            psb = pq.tile([P, 512], F32, tag="pq", name="psb")
            nc.tensor.matmul(psb[:], ew_sb[:], rr[:], start=True, stop=True)
            psbs = rs.tile([P, 512], F32, tag="psbs", name="psbs")
            nc.vector.tensor_copy(psbs[:], psb[:])
            for hh in range(2):
                b0 = 64 * hh
                asl = attn[w][b0:b0 + 64, 512 * ic:512 * ic + 512]
                nc.vector.tensor_mul(asl, po_w[hh][0:HD, :], psbs[b0:b0 + 64, :])

        ot_store = {}

        def emit_oproj_unit(sb, n4, tail=False):
            ss = slice(P * sb, P * sb + P)
            ps_o = pq.tile([P, 512], F32, tag="pq", name="ps_o")
            nc.tensor.matmul(
                ps_o[:], attn0[:, ss],
                wo_sb[:, 512 * n4:512 * n4 + 512],
                start=True, stop=False)
            nc.tensor.matmul(
                ps_o[:], attn1[:, ss],
                wo_sb[:, S + 512 * n4:S + 512 * n4 + 512],
                start=False, stop=True)
            if n4 == 0:
                ot_store[sb] = otp.tile([P, H], mm_dt, tag="otp", name="ot")
            ot = ot_store[sb]
            osl = slice(512 * n4, 512 * n4 + 512)
            if tail:
                # both scalar and DVE are idle at the tail: alternate casts
                if n4 % 2 == 0:
                    nc.scalar.copy(ot[:, osl], ps_o[:])
                else:
                    nc.vector.tensor_copy(ot[:, osl], ps_o[:])
            else:
                nc.vector.tensor_copy(ot[:, osl], ps_o[:])
            if n4 == NC4 - 1:
                nc.sync.dma_start(out_d[ss, :], ot_store.pop(sb)[:])

        # ---------------- emission ----------------
        proj_mm(0)
        rope(0)
        proj_mm(1)
        # chunk-0's first score groups (and their exps) are emitted before
        # rope(1) so the scalar engine starts the exp stream immediately
        # after the g=1 projection instead of behind rope(1)'s casts
        emit_scores(0, 0, 0)
        emit_scores(0, 1, 0)
        rope(1)
        pending_fin2 = None
        units = []
        ui = si = 0
        slots_total = 1

        for ic in range(NC4):
            nJ = 4 * ic + 4
            # filler units: o_proj of the previous chunk, spread over this
            # chunk's score/PV loop iterations (skipping the first two of
            # each wave, which cover fin2 / chunk-boundary latency)
            units = [(sb, n4) for sb in range(4 * (ic - 1), 4 * ic)
                     for n4 in range(NC4)] if ic > 0 else []
            ui = si = 0
            slots_total = max(1, 2 * (nJ - 5))
            for w in range(2):
                po_w = None
                npv = 0
                iters = list(range(2, nJ))
                # lookahead target: first two score groups of the next
                # wave/chunk, emitted mid-loop (long waves) so their exps
                # are already drained when the next wave's PVs need them
                if w == 0:
                    nxt = (ic, 1)
                elif ic + 1 < NC4:
                    nxt = (ic + 1, 0)
                else:
                    nxt = None
                inloop_la = len(iters) >= 6
                for idx, J in enumerate(iters):
                    emit_scores(ic, J, w)
                    if idx == 0 and pending_fin2 is not None:
                        fin2(*pending_fin2)
                        pending_fin2 = None
                    if idx >= 2:
                        if po_w is None:
                            po_w = [pq.tile([HD + 1, 512], F32, tag="pq",
                                            name=f"po{ic}_{w}_{hh}")
                                    for hh in range(2)]
                        emit_pv(ic, npv, w, po_w)
                        npv += 1
                        if inloop_la and nxt is not None and \
                                idx in (len(iters) - 4, len(iters) - 3):
                            emit_scores(nxt[0], idx - (len(iters) - 4), nxt[1])
                    # fillers skip the wave's last iteration so the
                    # DVE is free for the softmax-finalize chain
                    if 2 <= idx < len(iters) - 1:
                        si += 1
                        take = (len(units) * si) // slots_total - ui
                        while take > 0 and ui < len(units):
                            emit_oproj_unit(*units[ui])
                            ui += 1
                            take -= 1
                if po_w is None:
                    po_w = [pq.tile([HD + 1, 512], F32, tag="pq",
                                    name=f"po{ic}_{w}_{hh}")
                            for hh in range(2)]
                while npv < nJ:
                    emit_pv(ic, npv, w, po_w)
                    npv += 1
                fin1(ic, w, po_w)
                if nxt is not None and not inloop_la:
                    emit_scores(nxt[0], 0, nxt[1])
                    emit_scores(nxt[0], 1, nxt[1])
                pending_fin2 = (ic, w, po_w)
            while ui < len(units):
                emit_oproj_unit(*units[ui])
                ui += 1
        # tail: finalize the last wave, then its o_proj via scalar casts
        fin2(*pending_fin2)
        for sb in range(12, 16):
            for n4 in range(NC4):
                emit_oproj_unit(sb, n4, tail=True)

    nc.compile()
    return nc


_NC_CACHE = {}


def _get_module(mm_dt=MM_DT):
    if mm_dt not in _NC_CACHE:
        _NC_CACHE[mm_dt] = _build_module(mm_dt)
    return _NC_CACHE[mm_dt]


def _prep_inputs(x, wq, wk, wv, wo, cos, sin, mm_dt=MM_DT):
    mm_np = mybir.dt.np(mm_dt)
    x = np.asarray(x, dtype=np.float32)
    xT = np.ascontiguousarray(x.reshape(S, H).T.astype(mm_np))

    cosT = np.asarray(cos, dtype=np.float32).T          # [64, S]
    sinT = np.asarray(sin, dtype=np.float32).T          # [64, S]
    sgn = np.where(np.arange(HD) < HD // 2, -1.0, 1.0).astype(np.float32)
    sinT_s = sinT * sgn[:, None]
    cos2 = np.ascontiguousarray(np.tile(cosT, (2, 1))).astype(mm_np)  # [128, S]
    sin2 = np.ascontiguousarray(np.tile(sinT_s, (2, 1))).astype(mm_np)

    # inverted causal masks (1 where masked out), diagonal offsets 0..3,
    # in fp8e4 DoubleRow layout: [64 rows, (ktile two, 4 offsets, 512)]
    np8 = mybir.dt.np(mybir.dt.float8e4)
    jl = np.arange(P)[:, None]
    il = np.arange(512)[None, :]
    cm8 = np.zeros((64, 2, 4, 512), dtype=np.float32)
    for t in range(4):
        blk = (jl + P * t > il).astype(np.float32)
        cm8[:, 0, t, :] = blk[0:64]
        cm8[:, 1, t, :] = blk[64:128]
    cm8 = np.ascontiguousarray(cm8.reshape(64, 4096)).astype(np8)
    ni8 = np.zeros((64, 2, P), dtype=np.float32)
    for j in range(64):
        ni8[j, 0, j] = -MASK_NEG
        ni8[j, 1, 64 + j] = -MASK_NEG
    ni8 = np.ascontiguousarray(ni8.reshape(64, 2 * P)).astype(np8)

    # selector matrix: psb rows 0:64 get the reciprocal row of the wave's
    # first head (partition 0), rows 64:128 the second head (partition 32)
    ew = np.zeros((P, P), dtype=np.float32)
    ew[0, 0:64] = 1.0
    ew[32, 64:128] = 1.0
    ew = ew.astype(mm_np)

    def chunk_kxm(w):
        # [H, M] -> [128, KCH*M] with k-chunk-major free layout
        m = w.shape[1]
        return np.ascontiguousarray(
            w.reshape(KCH, P, m).transpose(1, 0, 2).reshape(P, KCH * m).astype(mm_np))

    wq = np.asarray(wq, dtype=np.float32)
    wk = np.asarray(wk, dtype=np.float32)
    wv = np.asarray(wv, dtype=np.float32)
    wo = np.asarray(wo, dtype=np.float32)

    in_maps = []
    for c in range(NCORES):
        wq_c = wq[:, DQ * c:DQ * c + DQ] * SCALE
        wkv_c = np.concatenate(
            [wk[:, HD * c:HD * c + HD], wv[:, HD * c:HD * c + HD]], axis=1)
        wo_c = wo[DQ * c:DQ * c + DQ, :]
        wo_l = np.ascontiguousarray(
            wo_c.reshape(2, P, H).transpose(1, 0, 2).reshape(P, 2 * H).astype(mm_np))
        in_maps.append({
            "xT": xT,
            "wq": chunk_kxm(wq_c),
            "wkv": chunk_kxm(wkv_c),
            "wo": wo_l,
            "cos2": cos2,
            "sin2": sin2,
            "cm8": cm8,
            "ni8": ni8,
            "ew": ew,
        })
    return in_maps


def run(inputs, trace=False, trace_kwargs=None, mm_dt=MM_DT):
    """Execute on 8 cores; returns (full_output, BassKernelResults)."""
    nc = _get_module(mm_dt)
    in_maps = _prep_inputs(
        inputs["x"], inputs["wq"], inputs["wk"], inputs["wv"],
        inputs["wo"], inputs["cos"], inputs["sin"], mm_dt=mm_dt)
    kwargs = {}
    if trace:
        kwargs = dict(trace=True, **(trace_kwargs or {}))
    res = run_bass_kernel_spmd(nc, in_maps, core_ids=list(range(NCORES)), **kwargs)
    acc = np.zeros((S, H), dtype=np.float32)
    for c in range(NCORES):
        acc += res.results[c]["out"].astype(np.float32)
    out = acc.reshape(1, S, H)
    return out, res


def kernel(**inputs):
    out, _ = run(inputs, trace=False)
    return out
